# revision 1
# baseline (speedup 1.0000x reference)
"""Trainium2 Bass kernel for a pre-norm transformer block (B=8, N=1024, C=768).

Strategy: data-parallel over batch — each of the 8 NeuronCores runs the full
block for one batch element. Activations are kept feature-major ([feat, tok])
so every matmul contracts over the partition dim with no on-device transposes:

  - LayerNorm over the partition (feature) dim via ones-vector PE matmuls for
    sum/sumsq, then GPSIMD partition_broadcast of per-token mean/rstd.
  - Attention scores computed transposed (S^T = K Q^T, keys on partitions);
    softmax without max-subtraction (logit range is ~[-1.6, 1.6] here), with
    the denominator fused into the P^T·V matmul as an extra ones-column of V.
  - V is produced token-major directly by swapping matmul operands
    (lhsT = h^T chunk, rhs = W_v), so it is already [keys, dk] for P^T·V.

All weights are pre-transposed/pre-tiled on the host into the exact SBUF
layouts, so every DMA is contiguous per partition.
"""

import os
import sys

import numpy as np

for _p in ("/opt/trn_rl_repo", "/root/.axon_site/_ro/trn_rl_repo"):
    if os.path.isdir(_p) and _p not in sys.path:
        sys.path.append(_p)

import concourse.bass as bass  # noqa: E402
import concourse.tile as tile  # noqa: E402
from concourse import bacc, mybir  # noqa: E402
from concourse.bass_utils import run_bass_kernel_spmd  # noqa: E402

F32 = mybir.dt.float32
F32R = mybir.dt.float32r
BF16 = mybir.dt.bfloat16

# "f32r" (full fp32 storage, TF32-like matmul) or "bf16" (bf16 activations +
# weights for matmuls; fp32 stats/residual/softmax-normalize paths).
MM_DTYPE = os.environ.get("KERNEL_MM_DTYPE", "f32r")

P = 128
D = 768
KD = D // P          # 6 subtiles over the 768 contraction dim
NTOK = 1024
F = 512              # token-half width (matmul free dim)
NHALF = NTOK // F    # 2
H = 12
DK = 64
DFF = 3072
MF1 = DFF // P       # 24
TC = NTOK // P       # 8 token chunks
EPS = 1e-5
N_CORES = 8


def _act_dt():
    # Tiles feeding matmuls carry the matmul dtype end-to-end: walrus's BIR
    # verifier requires FP32r matmul operands to be *produced* as FP32r.
    if MM_DTYPE == "bf16":
        return BF16
    if MM_DTYPE == "f32r":
        return F32R
    return F32


def _mm(ap):
    return ap


def build_program(reps=1):
    act = _act_dt()
    wdt = act
    nc = bacc.Bacc(
        "TRN2", target_bir_lowering=False, debug=False, num_devices=N_CORES
    )

    din = lambda name, shape, dt=F32: nc.dram_tensor(
        name, shape, dt, kind="ExternalInput"
    ).ap()
    # xt is typed f32r so LN-stat matmuls run at full PE rate; the DMA is a
    # bit-copy, so DVE consumers (normalize, residual) still see full fp32.
    xt = din("xt", [P, KD, NTOK], act if act != BF16 else F32)
    onesr = din("onesr", [P, 1], act if act != BF16 else F32)
    wqkv = din("wqkv", [18, P, KD, P], wdt)
    bqkv = din("bqkv", [P, 18])
    vbias = din("vbias", [P, KD, P])
    wproj = din("wproj", [KD, P, KD, P], wdt)
    bproj = din("bproj", [P, KD])
    ln1w = din("ln1w", [P, KD])
    ln1b = din("ln1b", [P, KD])
    ln2w = din("ln2w", [P, KD])
    ln2b = din("ln2b", [P, KD])
    wfc1 = din("wfc1", [MF1, P, KD, P], wdt)
    bfc1 = din("bfc1", [P, MF1])
    wfc2 = din("wfc2", [KD, P, MF1, P], wdt)
    bfc2 = din("bfc2", [P, KD])
    yt = nc.dram_tensor("yt", [P, KD, NTOK], F32, kind="ExternalOutput").ap()

    with tile.TileContext(nc) as tc:
        psum = tc.alloc_tile_pool(name="psum", bufs=8, space="PSUM")
        const = tc.alloc_tile_pool(name="const", bufs=1)
        stat = tc.alloc_tile_pool(name="stat", bufs=4)
        bcast = tc.alloc_tile_pool(name="bcast", bufs=3)
        tmp = tc.alloc_tile_pool(name="tmp", bufs=4)
        sqp = tc.alloc_tile_pool(name="sqp", bufs=4)
        outp = tc.alloc_tile_pool(name="outp", bufs=3)
        wstream = tc.alloc_tile_pool(name="wstream", bufs=6)

        def ps_tile():
            return psum.tile([P, F], F32, tag="ps", name="ps")

        # ---- constants ----
        ones_sb = const.tile([P, 1], F32)
        nc.vector.memset(ones_sb, 1.0)
        eps_sb = const.tile([P, 1], F32)
        nc.vector.memset(eps_sb, EPS)
        onesr_sb = const.tile([P, 1], onesr.dtype, name="onesr_sb")
        nc.sync.dma_start(out=onesr_sb[:], in_=onesr[:])

        def load_const(ap_dram, shape):
            t = const.tile(shape, ap_dram.dtype, name=ap_dram.name + "_sb")
            nc.sync.dma_start(out=t[:], in_=ap_dram[:])
            return t

        bqkv_sb = load_const(bqkv, [P, 18])
        vbias_sb = load_const(vbias, [P, KD, P])
        bproj_sb = load_const(bproj, [P, KD])
        ln1w_sb = load_const(ln1w, [P, KD])
        ln1b_sb = load_const(ln1b, [P, KD])
        ln2w_sb = load_const(ln2w, [P, KD])
        ln2b_sb = load_const(ln2b, [P, KD])
        bfc1_sb = load_const(bfc1, [P, MF1])
        bfc2_sb = load_const(bfc2, [P, KD])

        # ---- layernorm over the feature (partition) dim ----
        def layernorm(src_sb, w_sb, b_sb, dst_sb):
            # src/dst: [P, KD, NTOK]; stats per token via ones-matmuls.
            ones_for = onesr_sb if src_sb.dtype == F32R else ones_sb
            for half in range(NHALF):
                cols = slice(half * F, (half + 1) * F)
                sum_ps = ps_tile()
                for kk in range(KD):
                    nc.tensor.matmul(
                        sum_ps[0:1, :],
                        ones_for[:],
                        src_sb[:, kk, cols],
                        start=(kk == 0),
                        stop=(kk == KD - 1),
                    )
                sq_ps = ps_tile()
                for kk in range(KD):
                    sq = sqp.tile([P, F], src_sb.dtype, tag="sq", name="sq")
                    nc.vector.tensor_mul(sq[:], src_sb[:, kk, cols], src_sb[:, kk, cols])
                    nc.tensor.matmul(
                        sq_ps[0:1, :],
                        ones_for[:],
                        sq[:],
                        start=(kk == 0),
                        stop=(kk == KD - 1),
                    )
                mu = stat.tile([1, F], F32, tag="st", name="mu")
                nc.vector.tensor_scalar_mul(mu[:], sum_ps[0:1, :], 1.0 / D)
                mu_b = bcast.tile([P, F], F32, tag="bc", name="mu_b")
                nc.gpsimd.partition_broadcast(mu_b[:], mu[:])
                e2 = stat.tile([1, F], F32, tag="st", name="e2")
                nc.vector.tensor_scalar_mul(e2[:], sq_ps[0:1, :], 1.0 / D)
                var = stat.tile([1, F], F32, tag="st", name="var")
                nc.vector.tensor_mul(var[:], mu[:], mu[:])
                nc.vector.tensor_tensor(
                    var[:], e2[:], var[:], mybir.AluOpType.subtract
                )
                sd = stat.tile([1, F], F32, tag="st", name="sd")
                nc.scalar.activation(
                    sd[:], var[:], mybir.ActivationFunctionType.Sqrt,
                    bias=eps_sb[0:1], scale=1.0,
                )
                rs = stat.tile([1, F], F32, tag="st", name="rs")
                nc.vector.reciprocal(rs[:], sd[:])
                rs_b = bcast.tile([P, F], F32, tag="bc", name="rs_b")
                nc.gpsimd.partition_broadcast(rs_b[:], rs[:])
                for kk in range(KD):
                    t1 = tmp.tile([P, F], F32, tag="tmp", name="t1")
                    nc.vector.tensor_tensor(
                        t1[:], src_sb[:, kk, cols], mu_b[:], mybir.AluOpType.subtract
                    )
                    nc.vector.tensor_mul(t1[:], t1[:], rs_b[:])
                    nc.vector.tensor_scalar(
                        dst_sb[:, kk, cols], t1[:],
                        scalar1=w_sb[:, kk : kk + 1],
                        scalar2=b_sb[:, kk : kk + 1],
                        op0=mybir.AluOpType.mult,
                        op1=mybir.AluOpType.add,
                    )

        for _rep in range(reps):
            # =========== phase 0/1: load x, LN1 ===========
            # Pool alloc order is LIFO-constrained (stack allocator): longest-lived
            # pools first; MLP-era pools go on the right side of SBUF.
            xt_pool = tc.alloc_tile_pool(name="xt", bufs=1)
            attn_pool = tc.alloc_tile_pool(name="attn", bufs=1)
            qk_pool = tc.alloc_tile_pool(name="qk", bufs=6)
            vaug_pool = tc.alloc_tile_pool(name="vaug", bufs=1)
            h_pool = tc.alloc_tile_pool(name="h", bufs=1)
            wv_pool = tc.alloc_tile_pool(name="wv", bufs=1)

            xt_sb = xt_pool.tile([P, KD, NTOK], xt.dtype, name="xt_sb")
            for half in range(NHALF):
                for kk in range(KD):
                    cols = slice(half * F, (half + 1) * F)
                    nc.sync.dma_start(out=xt_sb[:, kk, cols], in_=xt[:, kk, cols])

            hT = h_pool.tile([P, KD, NTOK], act, name="hT")
            layernorm(xt_sb, ln1w_sb, ln1b_sb, hT)

            # =========== phase 2a: q/k projection chunks (feature-major) ===========
            # qk chunk m in 0..11 -> features m*128..m*128+127 of [q(768); k(768)].
            # Emitted as pairs (j, 6+j) interleaved with attention heads below, so
            # qk_pool only needs 6 bufs (3 pairs in flight).
            qk_tiles = {}

            def emit_qk_pair(j):
                for m in (j, KD + j):
                    wt = wstream.tile([P, KD, P], wdt, tag="w", name="wt")
                    nc.sync.dma_start(out=wt[:], in_=wqkv[m])
                    qkt = qk_pool.tile([P, NTOK], act, tag="qkt", name="qkt")
                    qk_tiles[m] = qkt
                    for half in range(NHALF):
                        cols = slice(half * F, (half + 1) * F)
                        ps = ps_tile()
                        for kk in range(KD):
                            nc.tensor.matmul(
                                ps[:],
                                wt[:, kk, :],
                                hT[:, kk, cols],
                                start=(kk == 0),
                                stop=(kk == KD - 1),
                            )
                        nc.vector.tensor_scalar(
                            qkt[:, cols], ps[:],
                            scalar1=bqkv_sb[:, m : m + 1],
                            scalar2=None,
                            op0=mybir.AluOpType.add,
                        )

            emit_qk_pair(0)
            emit_qk_pair(1)

            # =========== phase 2b: v projection (token-major) ===========
            wv_sb = wv_pool.tile([P, KD, KD, P], wdt, name="wv_sb")  # [p, kk, vm, o]
            nc.sync.dma_start(
                out=wv_sb[:], in_=wqkv[12:18].rearrange("m p kk o -> p kk m o")
            )
            v_aug = vaug_pool.tile([P, TC, H, DK + 1], act, name="v_aug")  # [tok_p, chunk, head, dk|1]
            nc.vector.tensor_copy(
                out=v_aug[:, :, :, DK : DK + 1],
                in_=ones_sb[:, None, None, :].to_broadcast([P, TC, H, 1]),
            )
            for t in range(TC):
                trange = slice(t * P, (t + 1) * P)
                ps2 = (ps_tile(), ps_tile())
                for kk in range(KD):
                    for nn in range(2):  # 512 + 256 of the 768 v features
                        nw = 512 if nn == 0 else 256
                        nc.tensor.matmul(
                            ps2[nn][:, :nw],
                            hT[:, kk, trange],
                            wv_sb[:, kk, 4 * nn : 4 * nn + nw // P, :],
                            start=(kk == 0),
                            stop=(kk == KD - 1),
                        )
                for nn in range(2):
                    nw = 512 if nn == 0 else 256
                    hw = nw // DK
                    nc.vector.tensor_tensor(
                        v_aug[:, t, nn * 8 : nn * 8 + hw, 0:DK],
                        ps2[nn][:, :nw].rearrange("p (h d) -> p h d", d=DK),
                        vbias_sb[:, 4 * nn : 4 * nn + nw // P, :].rearrange(
                            "p m o -> p (m o)"
                        ).rearrange("p (h d) -> p h d", d=DK),
                        mybir.AluOpType.add,
                    )

            wv_pool.release()

            # =========== phase 3: attention per head ===========
            pt_pool = tc.alloc_tile_pool(name="pt", bufs=6)
            attnT = attn_pool.tile([P, KD, NTOK], act, name="attnT")

            def emit_head_pair(j):
                # Heads 2j (partitions 0:64) and 2j+1 (64:128) of q/k chunk j.
                # Their K=64 score matmuls target disjoint PE row-groups and are
                # emitted adjacently so the array runs them concurrently.
                q_tile = qk_tiles[j]
                k_tile = qk_tiles[KD + j]
                pranges = (slice(0, DK), slice(DK, P))
                for half in range(NHALF):
                    cols = slice(half * F, (half + 1) * F)
                    o_ps = (ps_tile(), ps_tile())
                    for kc in range(TC):
                        pts = []
                        for hi in (0, 1):
                            pr = pranges[hi]
                            s_ps = ps_tile()
                            nc.tensor.matmul(
                                s_ps[:],
                                k_tile[pr, kc * P : (kc + 1) * P],
                                q_tile[pr, cols],
                                start=True,
                                stop=True,
                            )
                            pt = pt_pool.tile([P, F], act, tag="pt", name="pt")
                            nc.scalar.activation(
                                pt[:], s_ps[:], mybir.ActivationFunctionType.Exp,
                                scale=float(DK) ** -0.5,
                            )
                            pts.append(pt)
                        for hi in (0, 1):
                            nc.tensor.matmul(
                                o_ps[hi][0 : DK + 1, :],
                                v_aug[:, kc, 2 * j + hi, :],
                                pts[hi][:],
                                start=(kc == 0),
                                stop=(kc == TC - 1),
                            )
                    for hi in (0, 1):
                        rec = stat.tile([1, F], F32, tag="st", name="rec")
                        nc.vector.reciprocal(rec[:], o_ps[hi][DK : DK + 1, :])
                        rec_b = bcast.tile([DK, F], F32, tag="bc64", name="rec_b")
                        nc.gpsimd.partition_broadcast(rec_b[:], rec[:])
                        nc.vector.tensor_mul(
                            attnT[pranges[hi], j, cols], o_ps[hi][0:DK, :], rec_b[:]
                        )

            for j in range(KD):
                emit_head_pair(j)
                if j + 2 < KD:
                    emit_qk_pair(j + 2)

            pt_pool.release()
            h_pool.release()
            vaug_pool.release()
            qk_pool.release()

            # =========== phase 4: output projection + residual ===========
            x2_pool = tc.alloc_tile_pool(name="x2", bufs=1, side="right")
            x2T = x2_pool.tile([P, KD, NTOK], xt.dtype, name="x2T")
            wprojp = tc.alloc_tile_pool(name="wprojp", bufs=1)
            wp_sb = wprojp.tile([P, KD, KD, P], wdt, name="wp_sb")  # [p, kk, m, o]
            nc.sync.dma_start(
                out=wp_sb[:], in_=wproj[:].rearrange("m p kk o -> p kk m o")
            )
            for m in range(KD):
                ps2 = (ps_tile(), ps_tile())
                for kk in range(KD):
                    for half in range(NHALF):
                        cols = slice(half * F, (half + 1) * F)
                        nc.tensor.matmul(
                            ps2[half][:],
                            wp_sb[:, kk, m, :],
                            attnT[:, kk, cols],
                            start=(kk == 0),
                            stop=(kk == KD - 1),
                        )
                for half in range(NHALF):
                    cols = slice(half * F, (half + 1) * F)
                    nc.vector.tensor_scalar(
                        x2T[:, m, cols], ps2[half][:],
                        scalar1=bproj_sb[:, m : m + 1],
                        scalar2=None,
                        op0=mybir.AluOpType.add,
                    )
                    nc.vector.tensor_add(
                        x2T[:, m, cols], x2T[:, m, cols], xt_sb[:, m, cols]
                    )
            wprojp.release()

            attn_pool.release()
            xt_pool.release()

            # =========== phase 5: LN2 ===========
            h2_pool = tc.alloc_tile_pool(name="h2", bufs=1, side="right")
            h2T = h2_pool.tile([P, KD, NTOK], act, name="h2T")
            layernorm(x2T, ln2w_sb, ln2b_sb, h2T)

            # =========== phase 6: MLP ===========
            g_pool = tc.alloc_tile_pool(name="g", bufs=1, side="right")
            w2stream = tc.alloc_tile_pool(name="w2s", bufs=2, side="right")
            for half in range(NHALF):
                cols = slice(half * F, (half + 1) * F)
                gT = g_pool.tile([P, MF1, F], act, tag="g", name="gT")
                for m in range(MF1):
                    wt = wstream.tile([P, KD, P], wdt, tag="w", name="wt")
                    nc.sync.dma_start(out=wt[:], in_=wfc1[m])
                    ps = ps_tile()
                    for kk in range(KD):
                        nc.tensor.matmul(
                            ps[:],
                            _mm(wt[:, kk, :]),
                            _mm(h2T[:, kk, cols]),
                            start=(kk == 0),
                            stop=(kk == KD - 1),
                        )
                    nc.scalar.activation(
                        gT[:, m, :], ps[:], mybir.ActivationFunctionType.Gelu,
                        bias=bfc1_sb[:, m : m + 1], scale=1.0,
                    )
                for m in range(KD):
                    w2 = w2stream.tile([P, MF1, P], wdt, tag="w2", name="w2")
                    nc.sync.dma_start(out=w2[:], in_=wfc2[m])
                    ps = ps_tile()
                    for kk in range(MF1):
                        nc.tensor.matmul(
                            ps[:],
                            _mm(w2[:, kk, :]),
                            _mm(gT[:, kk, :]),
                            start=(kk == 0),
                            stop=(kk == MF1 - 1),
                        )
                    yo = outp.tile([P, F], F32, tag="yo", name="yo")
                    nc.vector.tensor_scalar(
                        yo[:], ps[:],
                        scalar1=bfc2_sb[:, m : m + 1],
                        scalar2=None,
                        op0=mybir.AluOpType.add,
                    )
                    nc.vector.tensor_add(yo[:], yo[:], x2T[:, m, cols])
                    nc.sync.dma_start(out=yt[:, m, cols], in_=yo[:])

            w2stream.release()
            g_pool.release()
            h2_pool.release()
            x2_pool.release()

        wstream.release()
        outp.release()
        sqp.release()
        tmp.release()
        bcast.release()
        stat.release()
        const.release()
        psum.release()

    nc.compile()
    return nc


def _retile_w(w_t, mtiles):
    """[out, in] torch-convention weight -> [mtiles, P, in//P, P] chunk layout.

    chunk[m, p, kk, o] = w_t[m*P + o, kk*P + p]
    """
    out_dim, in_dim = w_t.shape
    a = w_t.reshape(mtiles, P, in_dim // P, P).transpose(0, 3, 2, 1)
    return np.ascontiguousarray(a)


def _vec_tile(v):
    """[n] -> [P, n//P] with t[p, m] = v[m*P + p]."""
    return np.ascontiguousarray(v.reshape(-1, P).T)


_NC_CACHE = {}


def _get_nc():
    if "nc" not in _NC_CACHE:
        _NC_CACHE["nc"] = build_program()
    return _NC_CACHE["nc"]


def prep_inputs(x, ln1_w, ln1_b, qkv_w, qkv_b, proj_w, proj_b,
                ln2_w, ln2_b, fc1_w, fc1_b, fc2_w, fc2_b):
    wdt_np = np.dtype("float32")
    if MM_DTYPE == "bf16":
        import ml_dtypes

        wdt_np = np.dtype(ml_dtypes.bfloat16)

    f32 = lambda a: np.asarray(a, dtype=np.float32)
    x = f32(x)
    shared = {
        "onesr": np.ones((P, 1), dtype=np.float32),
        "wqkv": _retile_w(f32(qkv_w), 18).astype(wdt_np),
        "bqkv": _vec_tile(f32(qkv_b)),
        "vbias": np.ascontiguousarray(
            np.broadcast_to(f32(qkv_b)[1536:].reshape(1, KD, P), (P, KD, P))
        ),
        "wproj": _retile_w(f32(proj_w), KD).astype(wdt_np),
        "bproj": _vec_tile(f32(proj_b)),
        "ln1w": _vec_tile(f32(ln1_w)),
        "ln1b": _vec_tile(f32(ln1_b)),
        "ln2w": _vec_tile(f32(ln2_w)),
        "ln2b": _vec_tile(f32(ln2_b)),
        "wfc1": _retile_w(f32(fc1_w), MF1).astype(wdt_np),
        "bfc1": _vec_tile(f32(fc1_b)),
        "wfc2": _retile_w(f32(fc2_w), KD).astype(wdt_np),
        "bfc2": _vec_tile(f32(fc2_b)),
    }
    in_maps = []
    for b in range(N_CORES):
        m = dict(shared)
        # xt[p, s, n] = x[b, n, s*P + p]
        m["xt"] = np.ascontiguousarray(x[b].reshape(NTOK, KD, P).transpose(2, 1, 0))
        in_maps.append(m)
    return in_maps


def kernel(**inputs):
    nc = _get_nc()
    in_maps = prep_inputs(**inputs)
    res = run_bass_kernel_spmd(nc, in_maps, list(range(N_CORES)))
    outs = []
    for b in range(N_CORES):
        ytile = res.results[b]["yt"]  # [P, KD, NTOK]
        outs.append(ytile.transpose(2, 1, 0).reshape(NTOK, D))
    return np.stack(outs).astype(np.float32)



# revision 15
# speedup vs baseline: 1.1868x; 1.1868x over previous
"""Trainium2 Bass kernel for a pre-norm transformer block (B=8, N=1024, C=768).

Data-parallel over batch: each of 8 NeuronCores runs the full block for one
batch element.  Activations are feature-major ([feat, tok]) so every matmul
contracts over the partition dim with no on-device transposes.

v2 restructure (vs the straight-line v1):
  - LayerNorm gains are folded into the consuming weights on the host
    (W' = W@diag(ln_w), b' = b + W@ln_b), so LN emits only z=(x-mu)*rs.
  - The block is software-pipelined by token halves: attention for query
    half 1 (Activation-engine-bound softmax exp) runs with proj / LN2 / fc1
    of half 0 interleaved as PE filler work, so the PE never waits on exp.
  - fc1-half-0's gelu is deferred (DVE does the PSUM->SBUF bias add) so the
    Activation engine stays on the Exp table inside the overlap window
    (Exp and Gelu live in different act-function tables; a switch is 1.3us).
  - LN stat matmuls are fp32r (1 row/cycle at free=512); activations and
    weights are bf16 (same PE rate as fp32r, half the DMA and SBUF).
"""

import os
import sys

import numpy as np

for _p in ("/opt/trn_rl_repo", "/root/.axon_site/_ro/trn_rl_repo"):
    if os.path.isdir(_p) and _p not in sys.path:
        sys.path.append(_p)

import concourse.bass as bass  # noqa: E402
import concourse.tile as tile  # noqa: E402
from concourse import bacc, mybir  # noqa: E402
from concourse.bass_utils import run_bass_kernel_spmd  # noqa: E402

F32 = mybir.dt.float32
F32R = mybir.dt.float32r
BF16 = mybir.dt.bfloat16

P = 128
D = 768
KD = D // P          # 6 subtiles over the 768 contraction dim
NTOK = 1024
F = 512              # token-half width (matmul free dim)
NHALF = NTOK // F    # 2
H = 12
DK = 64
DFF = 3072
MF1 = DFF // P       # 24
TC = NTOK // P       # 8 token chunks
EPS = 1e-5
N_CORES = 8

ACT = BF16           # activation dtype for matmul operands
WDT = BF16           # weight dtype


def build_program(reps=1):
    nc = bacc.Bacc(
        "TRN2", target_bir_lowering=False, debug=False, num_devices=N_CORES
    )

    din = lambda name, shape, dt=F32: nc.dram_tensor(
        name, shape, dt, kind="ExternalInput"
    ).ap()
    # f32r: full fp32 storage; LN-stat matmuls run at 1 row/cycle.
    xt = din("xt", [P, KD, NTOK], F32R)
    onesr = din("onesr", [P, 1], F32R)
    wqkv = din("wqkv", [18, P, KD, P], WDT)
    bqkv = din("bqkv", [P, 18])
    vbias = din("vbias", [P, KD, P])
    wproj = din("wproj", [KD, P, KD, P], WDT)
    bproj = din("bproj", [P, KD])
    wfc1 = din("wfc1", [MF1, P, KD, P], WDT)
    bfc1 = din("bfc1", [P, MF1])
    wfc2 = din("wfc2", [KD, P, MF1, P], WDT)
    bfc2 = din("bfc2", [P, KD])
    yt = nc.dram_tensor("yt", [P, KD, NTOK], F32, kind="ExternalOutput").ap()

    EXP = mybir.ActivationFunctionType.Exp
    GELU = mybir.ActivationFunctionType.Gelu
    SQRT = mybir.ActivationFunctionType.Sqrt
    SUB = mybir.AluOpType.subtract
    ADD = mybir.AluOpType.add

    with tile.TileContext(nc) as tc:
        # ---- session pools ----
        psum = tc.alloc_tile_pool(name="psum", bufs=8, space="PSUM")
        const = tc.alloc_tile_pool(name="const", bufs=1)
        stat = tc.alloc_tile_pool(name="stat", bufs=4)
        bcast = tc.alloc_tile_pool(name="bcast", bufs=4)
        sqp = tc.alloc_tile_pool(name="sqp", bufs=2)
        outp = tc.alloc_tile_pool(name="outp", bufs=2)
        ptp = tc.alloc_tile_pool(name="ptp", bufs=4)
        wstream = tc.alloc_tile_pool(name="wstream", bufs=6)
        w2stream = tc.alloc_tile_pool(name="w2s", bufs=2, side="right")

        def ps_tile():
            return psum.tile([P, F], F32, tag="ps", name="ps")

        # ---- constants ----
        ones_sb = const.tile([P, 1], F32)
        nc.vector.memset(ones_sb, 1.0)
        eps_sb = const.tile([P, 1], F32)
        nc.vector.memset(eps_sb, EPS)
        onesr_sb = const.tile([P, 1], F32R, name="onesr_sb")
        nc.sync.dma_start(out=onesr_sb[:], in_=onesr[:])

        def load_const(ap_dram, shape):
            t = const.tile(shape, ap_dram.dtype, name=ap_dram.name + "_sb")
            nc.sync.dma_start(out=t[:], in_=ap_dram[:])
            return t

        bqkv_sb = load_const(bqkv, [P, 18])
        vbias_sb = load_const(vbias, [P, KD, P])
        bproj_sb = load_const(bproj, [P, KD])
        bfc1_sb = load_const(bfc1, [P, MF1])
        bfc2_sb = load_const(bfc2, [P, KD])

        # ---- layernorm helpers (z = (x - mu) * rs only; gains folded) ----
        def ln_stats(src_sb, half):
            cols = slice(half * F, (half + 1) * F)
            sum_ps = ps_tile()
            for kk in range(KD):
                nc.tensor.matmul(
                    sum_ps[0:1, :], onesr_sb[:], src_sb[:, kk, cols],
                    start=(kk == 0), stop=(kk == KD - 1),
                )
            sq_ps = ps_tile()
            for kk in range(KD):
                sq = sqp.tile([P, F], F32R, tag="sq", name="sq")
                nc.vector.tensor_mul(sq[:], src_sb[:, kk, cols], src_sb[:, kk, cols])
                nc.tensor.matmul(
                    sq_ps[0:1, :], onesr_sb[:], sq[:],
                    start=(kk == 0), stop=(kk == KD - 1),
                )
            mu = stat.tile([1, F], F32, tag="st", name="mu")
            nc.vector.tensor_scalar_mul(mu[:], sum_ps[0:1, :], 1.0 / D)
            mu_b = bcast.tile([P, F], F32, tag="bc", name="mu_b")
            nc.gpsimd.partition_broadcast(mu_b[:], mu[:])
            e2 = stat.tile([1, F], F32, tag="st", name="e2")
            nc.vector.tensor_scalar_mul(e2[:], sq_ps[0:1, :], 1.0 / D)
            var = stat.tile([1, F], F32, tag="st", name="var")
            nc.vector.tensor_mul(var[:], mu[:], mu[:])
            nc.vector.tensor_tensor(var[:], e2[:], var[:], SUB)
            sd = stat.tile([1, F], F32, tag="st", name="sd")
            nc.scalar.activation(sd[:], var[:], SQRT, bias=eps_sb[0:1], scale=1.0)
            rs = stat.tile([1, F], F32, tag="st", name="rs")
            nc.vector.reciprocal(rs[:], sd[:])
            rs_b = bcast.tile([P, F], F32, tag="bc", name="rs_b")
            nc.gpsimd.partition_broadcast(rs_b[:], rs[:])
            return mu_b, rs_b

        def ln_norm(src_sb, dst_sb, half, mu_b, rs_b, dst_local=False):
            cols = slice(half * F, (half + 1) * F)
            for kk in range(KD):
                d = dst_sb[:, kk, :] if dst_local else dst_sb[:, kk, cols]
                nc.vector.tensor_tensor(
                    d, src_sb[:, kk, cols], mu_b[:], SUB
                )
                nc.vector.tensor_mul(d, d, rs_b[:])

        for _rep in range(reps):
            # ---- per-rep pools (left) ----
            xt_pool = tc.alloc_tile_pool(name="xt", bufs=1)
            attn_pool = tc.alloc_tile_pool(name="attn", bufs=1)
            qk_pool = tc.alloc_tile_pool(name="qk", bufs=1)
            vaug_pool = tc.alloc_tile_pool(name="vaug", bufs=1)
            h_pool = tc.alloc_tile_pool(name="h", bufs=1)
            wv_pool = tc.alloc_tile_pool(name="wv", bufs=1)
            # ---- per-rep pools (right) ----
            x2_pool = tc.alloc_tile_pool(name="x2", bufs=1, side="right")
            h2_pool = tc.alloc_tile_pool(name="h2", bufs=1, side="right")
            g_pool = tc.alloc_tile_pool(name="g", bufs=1, side="right")

            xt_sb = xt_pool.tile([P, KD, NTOK], F32R, name="xt_sb")
            attnT = attn_pool.tile([P, KD, NTOK], ACT, name="attnT")
            # k chunks span all tokens; q chunks are per-half tiles (q-h1 all
            # six live through the overlap window, q-h0 transient).
            kT = qk_pool.tile([P, KD, NTOK], ACT, name="kT")
            v_aug = vaug_pool.tile([P, TC, H, DK + 1], ACT, name="v_aug")
            hT = h_pool.tile([P, KD, NTOK], ACT, name="hT")
            wv_sb = wv_pool.tile([P, KD, KD, P], WDT, name="wv_sb")
            x2T = x2_pool.tile([P, KD, NTOK], F32R, name="x2T")
            # h2 / g of the two halves share one slot each: h1's writes are
            # dep-serialized after h0's last reader, which matches the
            # schedule (fc1-h0 / fc2-h0 finish before the h1 tail runs).
            h2 = [
                h2_pool.tile([P, KD, F], ACT, tag="h2", name=f"h2_{i}")
                for i in range(2)
            ]
            gT0 = g_pool.tile([P, MF1, F], ACT, tag="g", name="gT0")
            gT1 = g_pool.tile([P, MF1, F], ACT, tag="g", name="gT1")

            # =========== prologue: load x, LN1, q/k/v ===========
            for half in range(NHALF):
                for kk in range(KD):
                    cols = slice(half * F, (half + 1) * F)
                    nc.sync.dma_start(out=xt_sb[:, kk, cols], in_=xt[:, kk, cols])

            # LN1 h0 stats, then h1 stats (PE), normalizes on DVE overlap qk.
            st0 = ln_stats(xt_sb, 0)
            st1 = ln_stats(xt_sb, 1)
            ln_norm(xt_sb, hT, 0, *st0)

            qk_w = {}
            q_tiles = {}

            def load_qk_w(j):
                wtq = wstream.tile([P, KD, P], WDT, tag="w", name="wtq")
                nc.sync.dma_start(out=wtq[:], in_=wqkv[j])
                wtk = wstream.tile([P, KD, P], WDT, tag="w", name="wtk")
                nc.sync.dma_start(out=wtk[:], in_=wqkv[KD + j])
                qk_w[j] = (wtq, wtk)

            def emit_k_chunk(j, half, wt):
                cols = slice(half * F, (half + 1) * F)
                ps = ps_tile()
                for kk in range(KD):
                    nc.tensor.matmul(
                        ps[:], wt[:, kk, :], hT[:, kk, cols],
                        start=(kk == 0), stop=(kk == KD - 1),
                    )
                nc.vector.tensor_scalar(
                    kT[:, j, cols], ps[:],
                    scalar1=bqkv_sb[:, KD + j : KD + j + 1], scalar2=None, op0=ADD,
                )

            def emit_q_chunk(j, half, wt):
                cols = slice(half * F, (half + 1) * F)
                ps = ps_tile()
                for kk in range(KD):
                    nc.tensor.matmul(
                        ps[:], wt[:, kk, :], hT[:, kk, cols],
                        start=(kk == 0), stop=(kk == KD - 1),
                    )
                qt = ptp.tile([P, F], ACT, tag=f"q{half}",
                              bufs=(3 if half == 0 else 6), name="qt")
                q_tiles[(j, half)] = qt
                nc.vector.tensor_scalar(
                    qt[:], ps[:],
                    scalar1=bqkv_sb[:, j : j + 1], scalar2=None, op0=ADD,
                )

            # v projection (token-major); chunk t uses only tokens of its half.
            nc.sync.dma_start(
                out=wv_sb[:], in_=wqkv[12:18].rearrange("m p kk o -> p kk m o")
            )
            nc.vector.tensor_copy(
                out=v_aug[:, :, :, DK : DK + 1],
                in_=ones_sb[:, None, None, :].to_broadcast([P, TC, H, 1]),
            )

            def emit_v_chunk(t):
                trange = slice(t * P, (t + 1) * P)
                ps2 = (ps_tile(), ps_tile())
                for kk in range(KD):
                    for nn in range(2):  # 512 + 256 of the 768 v features
                        nw = 512 if nn == 0 else 256
                        nc.tensor.matmul(
                            ps2[nn][:, :nw],
                            hT[:, kk, trange],
                            wv_sb[:, kk, 4 * nn : 4 * nn + nw // P, :],
                            start=(kk == 0), stop=(kk == KD - 1),
                        )
                for nn in range(2):
                    nw = 512 if nn == 0 else 256
                    hw = nw // DK
                    nc.vector.tensor_tensor(
                        v_aug[:, t, nn * 8 : nn * 8 + hw, 0:DK],
                        ps2[nn][:, :nw].rearrange("p (h d) -> p h d", d=DK),
                        vbias_sb[:, 4 * nn : 4 * nn + nw // P, :].rearrange(
                            "p m o -> p (m o)"
                        ).rearrange("p (h d) -> p h d", d=DK),
                        ADD,
                    )

            load_qk_w(0)
            load_qk_w(1)
            # PE order: k/q h0 (after norm h0), v h0 chunks, k h1, v h1 chunks.
            for j in (0, 1):
                emit_k_chunk(j, 0, qk_w[j][1])
                emit_q_chunk(j, 0, qk_w[j][0])
            ln_norm(xt_sb, hT, 1, *st1)
            for t in range(4):
                emit_v_chunk(t)
            for j in (0, 1):
                emit_k_chunk(j, 1, qk_w[j][1])
            for t in range(4, TC):
                emit_v_chunk(t)

            # =========== attention ===========
            def emit_head_pair(j, half, fillers=()):
                fillers = list(fillers)
                cols = slice(half * F, (half + 1) * F)
                pranges = (slice(0, DK), slice(DK, P))
                q_sb = q_tiles.pop((j, half))
                o_ps = (ps_tile(), ps_tile())
                for kc in range(TC):
                    pts = []
                    for hi in (0, 1):
                        pr = pranges[hi]
                        s_ps = ps_tile()
                        nc.tensor.matmul(
                            s_ps[:],
                            kT[pr, j, kc * P : (kc + 1) * P],
                            q_sb[pr, :],
                            start=True, stop=True,
                        )
                        pt = ptp.tile([P, F], ACT, tag="pt", name="pt")
                        nc.scalar.activation(
                            pt[:], s_ps[:], EXP, scale=float(DK) ** -0.5
                        )
                        pts.append(pt)
                    for hi in (0, 1):
                        nc.tensor.matmul(
                            o_ps[hi][0 : DK + 1, :],
                            v_aug[:, kc, 2 * j + hi, :],
                            pts[hi][:],
                            start=(kc == 0), stop=(kc == TC - 1),
                        )
                    if fillers:
                        fillers.pop(0)()
                while fillers:
                    fillers.pop(0)()
                for hi in (0, 1):
                    rec = stat.tile([1, F], F32, tag="st", name="rec")
                    nc.vector.reciprocal(rec[:], o_ps[hi][DK : DK + 1, :])
                    rec_b = bcast.tile([DK, F], F32, tag="bc64", bufs=2, name="rec_b")
                    nc.gpsimd.partition_broadcast(rec_b[:], rec[:])
                    nc.vector.tensor_mul(
                        attnT[pranges[hi], j, cols], o_ps[hi][0:DK, :], rec_b[:]
                    )

            wp_tiles = {}

            def load_wp(m):
                wp = wstream.tile([P, KD, P], WDT, tag="w", name="wp")
                nc.sync.dma_start(out=wp[:], in_=wproj[m])
                wp_tiles[m] = wp

            # ---- attn h0: head pairs with q/k chunk production as fillers ----
            for j in range(KD):
                fillers = [lambda j=j: emit_q_chunk(j, 1, qk_w[j][0])]
                if j + 2 < KD:
                    jj = j + 2

                    def _load_and_k0(jj=jj):
                        load_qk_w(jj)
                        emit_k_chunk(jj, 0, qk_w[jj][1])

                    fillers += [
                        _load_and_k0,
                        lambda jj=jj: emit_k_chunk(jj, 1, qk_w[jj][1]),
                        lambda jj=jj: emit_q_chunk(jj, 0, qk_w[jj][0]),
                    ]
                else:
                    # prefetch proj weights for the window
                    ms = range(3 * (j - 4), 3 * (j - 4) + 3)
                    fillers += [lambda m=m: load_wp(m) for m in ms]
                emit_head_pair(j, 0, fillers)

            # ---- window: attn h1 with proj/LN2/fc1 of h0 interleaved ----
            def emit_proj_chunk(m, half):
                cols = slice(half * F, (half + 1) * F)
                wp = wp_tiles.pop(m)
                ps = ps_tile()
                for kk in range(KD):
                    nc.tensor.matmul(
                        ps[:], wp[:, kk, :], attnT[:, kk, cols],
                        start=(kk == 0), stop=(kk == KD - 1),
                    )
                nc.vector.tensor_scalar(
                    x2T[:, m, cols], ps[:],
                    scalar1=bproj_sb[:, m : m + 1], scalar2=None, op0=ADD,
                )
                nc.vector.tensor_add(
                    x2T[:, m, cols], x2T[:, m, cols], xt_sb[:, m, cols]
                )

            def emit_fc1_chunk(m, half, deferred_gelu):
                wt = wstream.tile([P, KD, P], WDT, tag="w", name="wt")
                nc.sync.dma_start(out=wt[:], in_=wfc1[m])
                ps = ps_tile()
                for kk in range(KD):
                    nc.tensor.matmul(
                        ps[:], wt[:, kk, :], h2[half][:, kk, :],
                        start=(kk == 0), stop=(kk == KD - 1),
                    )
                gT = gT0 if half == 0 else gT1
                if deferred_gelu:
                    # DVE bias add; gelu applied post-window (keeps Act on Exp).
                    nc.vector.tensor_scalar(
                        gT[:, m, :], ps[:],
                        scalar1=bfc1_sb[:, m : m + 1], scalar2=None, op0=ADD,
                    )
                else:
                    nc.scalar.activation(
                        gT[:, m, :], ps[:], GELU,
                        bias=bfc1_sb[:, m : m + 1], scale=1.0,
                    )

            ln2_st = {}

            def ln2_stats_h0():
                ln2_st[0] = ln_stats(x2T, 0)

            def ln2_norm_h0():
                ln_norm(x2T, h2[0], 0, *ln2_st[0], dst_local=True)

            for j in range(KD):
                if j == 0:
                    fillers = [lambda m=m: emit_proj_chunk(m, 0) for m in range(KD)]
                elif j == 1:
                    fillers = [ln2_stats_h0, ln2_norm_h0]
                else:
                    fillers = [
                        lambda m=m: emit_fc1_chunk(m, 0, True)
                        for m in range(6 * (j - 2), 6 * (j - 2) + 6)
                    ]
                emit_head_pair(j, 1, fillers)

            # ---- post-window: gelu h0, proj h1, LN2 h1, fc2 h0, MLP h1 ----
            # zdep: a zero bias tile data-dependent on the last attention
            # output, pinning the deferred gelus after the exp window so the
            # scheduler can't interleave them (Exp/Gelu table thrash).
            zdep = stat.tile([P, 1], F32, tag="zdep", bufs=1, name="zdep")
            nc.vector.tensor_scalar_mul(zdep[:], attnT[:, KD - 1, NTOK - 1 : NTOK], 0.0)
            for m in range(KD):
                for mm in range(4 * m, 4 * m + 4):
                    nc.scalar.activation(
                        gT0[:, mm, :], gT0[:, mm, :], GELU,
                        bias=zdep[:, 0:1], scale=1.0,
                    )
                load_wp(m)
                emit_proj_chunk(m, 1)
            st2 = ln_stats(x2T, 1)
            ln_norm(x2T, h2[1], 1, *st2, dst_local=True)

            def emit_fc2_chunk(m, half):
                cols = slice(half * F, (half + 1) * F)
                gT = gT0 if half == 0 else gT1
                w2 = w2stream.tile([P, MF1, P], WDT, tag="w2", name="w2")
                nc.sync.dma_start(out=w2[:], in_=wfc2[m])
                ps = ps_tile()
                for kk in range(MF1):
                    nc.tensor.matmul(
                        ps[:], w2[:, kk, :], gT[:, kk, :],
                        start=(kk == 0), stop=(kk == MF1 - 1),
                    )
                yo = outp.tile([P, F], F32, tag="yo", name="yo")
                nc.vector.tensor_scalar(
                    yo[:], ps[:],
                    scalar1=bfc2_sb[:, m : m + 1], scalar2=None, op0=ADD,
                )
                nc.vector.tensor_add(yo[:], yo[:], x2T[:, m, cols])
                nc.sync.dma_start(out=yt[:, m, cols], in_=yo[:])

            for m in range(KD):
                emit_fc2_chunk(m, 0)
            for m in range(MF1):
                emit_fc1_chunk(m, 1, False)
            for m in range(KD):
                emit_fc2_chunk(m, 1)

            g_pool.release()
            h2_pool.release()
            x2_pool.release()
            wv_pool.release()
            h_pool.release()
            vaug_pool.release()
            qk_pool.release()
            attn_pool.release()
            xt_pool.release()

        w2stream.release()
        wstream.release()
        ptp.release()
        outp.release()
        sqp.release()
        bcast.release()
        stat.release()
        const.release()
        psum.release()

    nc.compile()
    return nc


def _retile_w(w_t, mtiles):
    """[out, in] torch-convention weight -> [mtiles, P, in//P, P] chunk layout.

    chunk[m, p, kk, o] = w_t[m*P + o, kk*P + p]
    """
    out_dim, in_dim = w_t.shape
    a = w_t.reshape(mtiles, P, in_dim // P, P).transpose(0, 3, 2, 1)
    return np.ascontiguousarray(a)


def _vec_tile(v):
    """[n] -> [P, n//P] with t[p, m] = v[m*P + p]."""
    return np.ascontiguousarray(v.reshape(-1, P).T)


_NC_CACHE = {}


def _get_nc():
    if "nc" not in _NC_CACHE:
        _NC_CACHE["nc"] = build_program()
    return _NC_CACHE["nc"]


def prep_inputs(x, ln1_w, ln1_b, qkv_w, qkv_b, proj_w, proj_b,
                ln2_w, ln2_b, fc1_w, fc1_b, fc2_w, fc2_b):
    import ml_dtypes

    wdt_np = np.dtype(ml_dtypes.bfloat16)
    f32 = lambda a: np.asarray(a, dtype=np.float32)
    x = f32(x)

    # Fold LN gains/biases into the consuming weights.
    qkv_w_f = f32(qkv_w) * f32(ln1_w)[None, :]
    qkv_b_f = f32(qkv_b) + f32(qkv_w) @ f32(ln1_b)
    fc1_w_f = f32(fc1_w) * f32(ln2_w)[None, :]
    fc1_b_f = f32(fc1_b) + f32(fc1_w) @ f32(ln2_b)

    shared = {
        "onesr": np.ones((P, 1), dtype=np.float32),
        "wqkv": _retile_w(qkv_w_f, 18).astype(wdt_np),
        "bqkv": _vec_tile(qkv_b_f),
        "vbias": np.ascontiguousarray(
            np.broadcast_to(qkv_b_f[1536:].reshape(1, KD, P), (P, KD, P))
        ),
        "wproj": _retile_w(f32(proj_w), KD).astype(wdt_np),
        "bproj": _vec_tile(f32(proj_b)),
        "wfc1": _retile_w(fc1_w_f, MF1).astype(wdt_np),
        "bfc1": _vec_tile(fc1_b_f),
        "wfc2": _retile_w(f32(fc2_w), KD).astype(wdt_np),
        "bfc2": _vec_tile(f32(fc2_b)),
    }
    in_maps = []
    for b in range(N_CORES):
        m = dict(shared)
        # xt[p, s, n] = x[b, n, s*P + p]
        m["xt"] = np.ascontiguousarray(x[b].reshape(NTOK, KD, P).transpose(2, 1, 0))
        in_maps.append(m)
    return in_maps


def kernel(**inputs):
    nc = _get_nc()
    in_maps = prep_inputs(**inputs)
    res = run_bass_kernel_spmd(nc, in_maps, list(range(N_CORES)))
    outs = []
    for b in range(N_CORES):
        ytile = res.results[b]["yt"]  # [P, KD, NTOK]
        outs.append(ytile.transpose(2, 1, 0).reshape(NTOK, D))
    return np.stack(outs).astype(np.float32)
